# revision 51
# baseline (speedup 1.0000x reference)
"""Trainium2 Bass kernel for GCACULanguageAwareAdapter.

Model (per example):
    var  = windowed variance of x over S (window 15, zero-padded)
    gate = sigmoid(tanh([x, var] @ Wg1 + bg1) @ Wg2 + bg2)       [B,S,H]
    proj = tanh(x @ Wd1 + bd1) @ Wd2 + bd2
    adapted = x + 0.1 * gate * proj
    logits  = adapted @ Wcls + bcls                               [B,S,L]
    returns (logits, gate)

Sharding: data-parallel over batch, one example per NeuronCore (B=8, 8 cores).

On-chip strategy (per core, x is [S=4096, H=768]):
  - token-major tiles [128 tok, 768 feat] ("A" layout) feed banded matmuls on
    the TensorEngine that compute the sliding-window sums:
        wsum1 = Band @ x,  wsum2 = Band @ x^2     (Band entries are exactly 1.0)
    Band is split into 3 constant pieces (prev-halo [7,128], cur [128,128],
    next-halo [7,128]) so operands never span SBUF tiles.
  - var*225 = 15*wsum2 - wsum1^2 computed on ACT (square) + DVE (fused STT);
    the 1/225 is folded into the bottom half of Wg1 host-side, 0.1 into Wd2.
  - weight matmuls contract features, so activations are transposed to
    feature-major ("B" layout) via PE transposes (identity moving operand),
    batched 4-per-PSUM-bank with single escape copies.
  - all matmul operands are bf16 (fp32 PE is 4x slower); elementwise psum math
    stays fp32.
"""

import numpy as np
import ml_dtypes

import concourse.bass as bass
import concourse.mybir as mybir
import concourse.tile as tile
from concourse.bass_utils import run_bass_kernel_spmd
from concourse.tile_scheduler import DMAInst


def legalize_waits(nc):
    """Spill excess sync waits onto EventSemaphore carrier instructions.

    The walrus codegen in this container accepts at most 1 sync-wait per
    regular instruction (2 for EventSemaphore), but Tile emits on_wait lists
    of arbitrary length ("Too many sync wait commands" codegen error).
    Semantically-equivalent fix: park extra waits on EVSEM instructions
    inserted just before, on the same engine queue.
    """
    for f in nc.m.functions:
        for bb in f.blocks:
            insts = bb.instructions
            out = []
            changed = False
            for inst in insts:
                si = inst.sync_info
                ow = list(si.on_wait) if si is not None and si.on_wait else []
                cap = 2 if isinstance(inst, mybir.InstEventSemaphore) else 1
                if len(ow) > cap and inst.engine != mybir.EngineType.Unassigned:
                    spill, keep = ow[:-1], ow[-1:]
                    for j in range(0, len(spill), 2):
                        ev = mybir.InstEventSemaphore(
                            name=f"{inst.name}-wsp{j}",
                            engine=inst.engine,
                            ins=[], outs=[],
                            bass_nofuse=True,
                            sync_info=mybir.SyncInfo(
                                on_wait=spill[j:j + 2], on_update=[]),
                        )
                        out.append(ev)
                    inst.sync_info = mybir.SyncInfo(
                        on_wait=keep, on_update=list(si.on_update))
                    changed = True
                out.append(inst)
            if changed:
                bb.instructions = out

BF16 = mybir.dt.bfloat16
F32 = mybir.dt.float32
NPBF16 = ml_dtypes.bfloat16
FP8 = mybir.dt.float8e4
NPFP8 = ml_dtypes.float8_e4m3

S, H, D, L = 4096, 768, 256, 2
W = 7
KW = 2 * W + 1          # 15
P = 128                 # tokens per tile
N_CORES = 8
GROUP = 4               # token tiles per group (512 tokens)
# 768-wide outputs are split into PSUM-bank-sized N slices
NSL = [(0, 0, 384), (1, 384, 384)]   # (bank_idx, offset, width)


def _band_pieces():
    """Constant banded matrices (entries exactly 1.0) for the window sums.

    wsum[t] = sum_{tau: |t - tau| <= W} x[tau], computed per 128-token tile as
    cur-piece + prev-halo + next-halo contributions.  band[tau_local, t_local].
    """
    r = np.arange(P)[:, None]
    t = np.arange(P)[None, :]
    # piece A rides on the 7-left-shifted tile (rows = tokens t0-7 .. t0+120),
    # piece B on the NEXT tile's shifted tile (rows = tokens t0+121 ..).
    a = (np.abs(t - (r - W)) <= W).astype(np.float32)
    b = (np.abs(t - (r + P - W)) <= W).astype(np.float32)
    return a, b


def build_bass(s_len=S, add_bg2=False, add_bd2=False, legalize=True):
    """Build the per-core Bass module. s_len must be a multiple of 512."""
    nt = s_len // P
    ng = s_len // (P * GROUP)
    nc = bass.Bass()

    # padded by W=7 zero rows on top, 121 on the bottom: row r = token r-7.
    x8_d = nc.dram_tensor("xpad8", [s_len + P, H], FP8, kind="ExternalInput")
    xT_d = nc.dram_tensor("xT", [H, s_len], BF16, kind="ExternalInput")
    xsq8_d = nc.dram_tensor("xsqpad8", [s_len + P, H], FP8, kind="ExternalInput")
    wg1_d = nc.dram_tensor("wg1", [12, P, D], BF16, kind="ExternalInput")
    wg2_d = nc.dram_tensor("wg2", [2, P, H], BF16, kind="ExternalInput")
    wd1_d = nc.dram_tensor("wd1", [6, P, D // 2], BF16, kind="ExternalInput")
    wd2_d = nc.dram_tensor("wd2", [P, H], BF16, kind="ExternalInput")
    wcls_d = nc.dram_tensor("wcls", [6, P, L], BF16, kind="ExternalInput")
    bg1_d = nc.dram_tensor("bg1", [P, 2], F32, kind="ExternalInput")
    bd1_d = nc.dram_tensor("bd1", [P, 1], F32, kind="ExternalInput")
    bcls_d = nc.dram_tensor("bcls", [L, 1], F32, kind="ExternalInput")
    bg2_d = bd2_d = None
    if add_bg2:
        bg2_d = nc.dram_tensor("bg2", [1, H], BF16, kind="ExternalInput")
    if add_bd2:
        bd2_d = nc.dram_tensor("bd2", [1, H], BF16, kind="ExternalInput")

    gate_d = nc.dram_tensor("gate_out", [s_len, H], F32, kind="ExternalOutput")
    logit_d = nc.dram_tensor("logits_out", [s_len, L], F32, kind="ExternalOutput")

    ba, bb = _band_pieces()
    # fp8 DoubleRow: the two shifted pieces stack on the pair axis and
    # contract in a single K=256 matmul.  x^2 goes through a 15x-scaled band
    # so var*225 = wsum2' - wsum1^2 is a plain subtract.
    bab = np.stack([ba, bb], axis=1)            # [128, 2, 128]
    band_d = nc.inline_tensor(bab.astype(NPFP8), "band_ab")
    b15_d = nc.inline_tensor((15 * bab).astype(NPFP8), "b15_ab")
    ident_d = nc.inline_tensor(np.eye(P, dtype=NPBF16), "ident")

    with tile.TileContext(nc) as tc:
        with (
            tc.tile_pool(name="const", bufs=1) as cpool,
            tc.tile_pool(name="xp", bufs=10) as xpool,
            tc.tile_pool(name="sqp", bufs=8) as sqpool,
            tc.tile_pool(name="vp", bufs=8) as vpool,
            tc.tile_pool(name="mp", bufs=4) as mpool,
            tc.tile_pool(name="bp", bufs=3) as bpool,
            tc.tile_pool(name="actp", bufs=3) as actpool,
            tc.tile_pool(name="gatep", bufs=5) as gatepool,
            tc.tile_pool(name="adp", bufs=8) as adpool,
            tc.tile_pool(name="psp", bufs=3, space=bass.MemorySpace.PSUM) as psp,
        ):
            # ---- constants into SBUF ----
            cn = [0]

            def const_tile(shape, dtype, src, engine=None):
                t = cpool.tile(shape, dtype, tag=f"c{cn[0]}")
                cn[0] += 1
                (engine or nc.gpsimd).dma_start(out=t[:], in_=src)
                return t

            band_sb = const_tile([P, 2, P], FP8, band_d[:])
            b15_sb = const_tile([P, 2, P], FP8, b15_d[:])
            ident_sb = const_tile([P, P], BF16, ident_d[:])
            # split the big weight loads across DMA queues (startup latency)
            wg1_sb = cpool.tile([P, 12, D], BF16, tag="wg1")
            for k in range(4):
                nc.gpsimd.dma_start(
                    out=wg1_sb[:, 3 * k:3 * k + 3, :],
                    in_=wg1_d[3 * k:3 * k + 3].rearrange("k p n -> p k n"))
            wg2_sb = cpool.tile([P, 2, H], BF16, tag="wg2")
            for k in range(2):
                nc.gpsimd.dma_start(
                    out=wg2_sb[:, k:k + 1, :],
                    in_=wg2_d[k:k + 1].rearrange("k p n -> p k n"))
            wd1_sb = const_tile([P, 6, D // 2], BF16,
                                wd1_d[:].rearrange("k p n -> p k n"))
            wd2_sb = const_tile([P, H], BF16, wd2_d[:])
            wcls_sb = const_tile([P, 6, L], BF16, wcls_d[:].rearrange("k p n -> p k n"))
            bg1_sb = const_tile([P, 2], F32, bg1_d[:])
            bd1_sb = const_tile([P, 1], F32, bd1_d[:])
            bcls_sb = const_tile([L, 1], F32, bcls_d[:])
            bg2_sb = const_tile([1, H], BF16, bg2_d[:]) if add_bg2 else None
            bd2_sb = const_tile([1, H], BF16, bd2_d[:]) if add_bd2 else None
            ones_sb = None
            if add_bg2 or add_bd2:
                ones_sb = cpool.tile([1, s_len], BF16)
                nc.vector.memset(ones_sb[:], 1.0)

            xs_t = {}     # tile idx -> x shifted left by 7 tokens
            qs_t = {}     # tile idx -> x^2 shifted left by 7 tokens
            var_t = {}    # tile idx -> var*225 bf16
            ad_t = {}     # tile idx (in group) -> adapted bf16
            esc_n = [0]   # escape-copy engine round robin

            def load_pair(i):
                """Paired shifted tiles [128, 2, H] fp8: pair slot j holds
                rows [128(i+j)-7, 128(i+j)+121) of x / x^2 (zero-padded)."""
                if i >= nt or i in xs_t:
                    return
                for (dram, store, tg) in ((x8_d, xs_t, "xs"), (xsq8_d, qs_t, "qs")):
                    t = sqpool.tile([P, 2, H], FP8, tag=tg)
                    nc.sync.dma_start(
                        out=t[:],
                        in_=dram[i * P:(i + 2) * P, :].rearrange(
                            "(j p) h -> p j h", j=2))
                    store[i] = t

            def escape(dst_view, src_view, scale=None):
                """PSUM -> SBUF copy alternating ACT/DVE."""
                if esc_n[0] % 3 == 0:
                    if scale is None:
                        nc.scalar.copy(dst_view, src_view)
                    else:
                        nc.scalar.mul(dst_view, src_view, scale)
                else:
                    if scale is None:
                        nc.vector.tensor_copy(dst_view, src_view)
                    else:
                        nc.vector.tensor_scalar_mul(dst_view, src_view, scale)
                esc_n[0] += 1

            def transpose_round(tag, srcs, dtype=BF16, scale=None):
                """Transpose 4 token tiles into one B-layout sbuf tile
                [128 feat, 6 fc, 512 tok]; 8 transposes fill one bf16 PSUM
                bank -> single escape copy per fc pair."""
                bt = bpool.tile([P, 6, GROUP * P], dtype, tag=tag)
                for fp in range(3):
                    ps = psp.tile([P, 2, GROUP, P], BF16, tag="ps1", bufs=2)
                    for k in range(2):
                        fc = fp * 2 + k
                        for ti in range(GROUP):
                            nc.tensor.transpose(
                                ps[:, k, ti, :],
                                srcs[ti][:, fc * P:(fc + 1) * P], ident_sb[:])
                    escape(
                        bt[:, fp * 2:fp * 2 + 2, :].rearrange(
                            "p c (a b) -> p c a b", a=GROUP), ps[:], scale)
                return bt

            for k in range(GROUP):
                load_pair(k)

            for g in range(ng):
                i0 = g * GROUP
                # ---------- per-tile: window sums + var ----------
                for ti in range(GROUP):
                    i = i0 + ti
                    load_pair(i)
                    w1 = psp.tile([P, 2, 512], F32, tag="ps2")
                    w2 = psp.tile([P, 2, 512], F32, tag="ps2")
                    DR = mybir.MatmulPerfMode.DoubleRow
                    for j, off, wd in NSL:
                        nc.tensor.matmul(w1[:, j, :wd], band_sb[:],
                                         xs_t[i][:, :, off:off + wd],
                                         start=True, stop=True, perf_mode=DR)
                        nc.tensor.matmul(w2[:, j, :wd], b15_sb[:],
                                         qs_t[i][:, :, off:off + wd],
                                         start=True, stop=True, perf_mode=DR)
                    msq = mpool.tile([P, H], BF16, tag="msq")
                    vt = vpool.tile([P, H], BF16, tag="var")
                    mv = msq[:].rearrange("p (a b) -> p a b", a=2)
                    nc.scalar.square(mv, w1[:, :, 0:384])
                    # var*225 = wsum2' - wsum1^2   (15x folded into band)
                    nc.vector.tensor_tensor(
                        vt[:].rearrange("p (a b) -> p a b", a=2),
                        w2[:, :, 0:384], mv, op=mybir.AluOpType.subtract)
                    var_t[i] = vt

                # ---------- group: B-layout operands ----------
                xB = bpool.tile([P, 6, GROUP * P], BF16, tag="xB")
                nc.sync.dma_start(
                    out=xB[:],
                    in_=xT_d[:].rearrange("(k p) s -> p k s", p=P)[
                        :, :, i0 * P:(i0 + GROUP) * P])
                vB = transpose_round("vB", [var_t[i0 + k] for k in range(GROUP)])

                # ---------- group: g1 -> t1B [256, 512], tanh ----------
                t1ps = psp.tile([P, 2, 512], F32, tag="ps2")
                for kc in range(12):
                    src = xB[:, kc, :] if kc < 6 else vB[:, kc - 6, :]
                    for dc in range(2):
                        nc.tensor.matmul(t1ps[:, dc, :],
                                         wg1_sb[:, kc, dc * P:(dc + 1) * P],
                                         src, start=kc == 0, stop=kc == 11)
                t1sb = actpool.tile([P, 2, 512], BF16, tag="t1")
                for dc in range(2):
                    nc.scalar.activation(t1sb[:, dc, :], t1ps[:, dc, :],
                                         mybir.ActivationFunctionType.Tanh,
                                         bias=bg1_sb[:, dc:dc + 1], scale=1.0)

                # ---------- group: d1 -> p1B [128, 512], tanh ----------
                p1ps = psp.tile([P, 512], F32, tag="ps1", bufs=2)
                for kc in range(6):
                    nc.tensor.matmul(p1ps[:], wd1_sb[:, kc, :], xB[:, kc, :],
                                     start=kc == 0, stop=kc == 5)
                p1sb = actpool.tile([P, 512], BF16, tag="p1")
                nc.scalar.activation(p1sb[:], p1ps[:],
                                     mybir.ActivationFunctionType.Tanh,
                                     bias=bd1_sb[:], scale=1.0)

                # ---------- per-tile: g2 -> gate, d2 -> proj, adapted ----------
                for ti in range(GROUP):
                    i = i0 + ti
                    tsl = slice(ti * P, (ti + 1) * P)
                    gps = psp.tile([P, 2, 512], F32, tag="ps2")
                    for dc in range(2):
                        for j, off, wd in NSL:
                            nc.tensor.matmul(gps[:, j, :wd], t1sb[:, dc, tsl],
                                             wg2_sb[:, dc, off:off + wd],
                                             start=dc == 0,
                                             stop=dc == 1 and bg2_sb is None)
                    if bg2_sb is not None:
                        for j, off, wd in NSL:
                            nc.tensor.matmul(gps[:, j, :wd], ones_sb[:, tsl],
                                             bg2_sb[:, off:off + wd],
                                             start=False, stop=True)
                    gate = gatepool.tile([P, H], F32, tag="gate")
                    nc.scalar.activation(
                        gate[:].rearrange("p (a b) -> p a b", a=2),
                        gps[:, :, 0:384],
                        mybir.ActivationFunctionType.Sigmoid)
                    nc.sync.dma_start(out=gate_d[i * P:(i + 1) * P, :], in_=gate[:])

                    pps = psp.tile([P, 2, 512], F32, tag="ps2")
                    for j, off, wd in NSL:
                        nc.tensor.matmul(pps[:, j, :wd], p1sb[:, tsl],
                                         wd2_sb[:, off:off + wd],
                                         start=True, stop=bd2_sb is None)
                    if bd2_sb is not None:
                        for j, off, wd in NSL:
                            nc.tensor.matmul(pps[:, j, :wd], ones_sb[:, tsl],
                                             bd2_sb[:, off:off + wd],
                                             start=False, stop=True)
                    gp = adpool.tile([P, H], BF16, tag="gp")
                    nc.vector.tensor_tensor(
                        gp[:].rearrange("p (a b) -> p a b", a=2),
                        gate[:].rearrange("p (a b) -> p a b", a=2),
                        pps[:, :, 0:384], op=mybir.AluOpType.mult)
                    ad_t[i] = gp

                # ---------- group: adaptedT = gpT + xT, cls -> logits ----------
                gB = transpose_round("gB", [ad_t[i0 + k] for k in range(GROUP)])
                aB = bpool.tile([P, 6, GROUP * P], BF16, tag="aB")
                nc.vector.tensor_tensor(aB[:], gB[:], xB[:],
                                        op=mybir.AluOpType.add)
                lps = psp.tile([L, 512], F32, tag="ps1", bufs=2)
                for kc in range(6):
                    nc.tensor.matmul(lps[:], wcls_sb[:, kc, :], aB[:, kc, :],
                                     start=kc == 0, stop=kc == 5)
                lsb = mpool.tile([L, 512], F32, tag="lsb")
                nc.scalar.add(lsb[:], lps[:], bcls_sb[:])
                nc.sync.dma_start(
                    out=logit_d[i0 * P:(i0 + GROUP) * P, :].rearrange("t c -> c t"),
                    in_=lsb[:])

                # drop refs that are no longer needed (frees pool slots)
                for ti in range(GROUP):
                    i = i0 + ti
                    var_t.pop(i, None)
                    ad_t.pop(i, None)
                    xs_t.pop(i, None)
                    qs_t.pop(i, None)

    if legalize:
        legalize_waits(nc)
    return nc


_CACHE = {}


def _get_nc(s_len, add_bg2, add_bd2):
    key = (s_len, add_bg2, add_bd2)
    if key not in _CACHE:
        _CACHE[key] = build_bass(s_len, add_bg2, add_bd2)
    return _CACHE[key]


def prep_weights(W_g1, b_g1, W_g2, b_g2, W_d1, b_d1, W_d2, b_d2, W_cls, b_cls):
    """Host-side weight prep: bf16 casts, layout chunking, scale folding."""
    wg1 = np.concatenate([W_g1[:H], W_g1[H:] / 225.0], axis=0)  # [1536, 256]
    wg1 = wg1.reshape(12, P, D).astype(NPBF16)
    wg2 = W_g2.reshape(2, P, H).astype(NPBF16)
    wd1 = W_d1.reshape(6, P, D // 2).astype(NPBF16)
    wd2 = (0.1 * W_d2).reshape(P, H).astype(NPBF16)
    wcls = W_cls.reshape(6, P, L).astype(NPBF16)
    inp = {
        "wg1": wg1, "wg2": wg2, "wd1": wd1, "wd2": wd2, "wcls": wcls,
        "bg1": np.ascontiguousarray(b_g1.reshape(2, P).T).astype(np.float32),
        "bd1": b_d1.reshape(P, 1).astype(np.float32),
        "bcls": b_cls.reshape(L, 1).astype(np.float32),
    }
    add_bg2 = bool(np.any(b_g2))
    add_bd2 = bool(np.any(b_d2))
    if add_bg2:
        inp["bg2"] = b_g2.reshape(1, H).astype(NPBF16)
    if add_bd2:
        inp["bd2"] = b_d2.reshape(1, H).astype(NPBF16)
    return inp, add_bg2, add_bd2


def kernel(sequence_output, W_g1, b_g1, W_g2, b_g2, W_d1, b_d1, W_d2, b_d2,
           W_cls, b_cls):
    x = np.asarray(sequence_output, np.float32)
    B, s_len, _ = x.shape
    winp, add_bg2, add_bd2 = prep_weights(
        np.asarray(W_g1, np.float32), np.asarray(b_g1, np.float32),
        np.asarray(W_g2, np.float32), np.asarray(b_g2, np.float32),
        np.asarray(W_d1, np.float32), np.asarray(b_d1, np.float32),
        np.asarray(W_d2, np.float32), np.asarray(b_d2, np.float32),
        np.asarray(W_cls, np.float32), np.asarray(b_cls, np.float32))
    nc = _get_nc(s_len, add_bg2, add_bd2)

    in_maps = []
    for b in range(B):
        m = dict(winp)
        xb = x[b].astype(NPBF16)
        xpad = np.zeros((s_len + P, H), NPBF16)
        xpad[W:W + s_len] = xb
        m["xpad8"] = xpad.astype(NPFP8)
        m["xT"] = np.ascontiguousarray(xb.T)
        sqpad = np.zeros((s_len + P, H), np.float32)
        sqpad[W:W + s_len] = xb.astype(np.float32) ** 2
        m["xsqpad8"] = sqpad.astype(NPFP8)
        in_maps.append(m)

    res = run_bass_kernel_spmd(nc, in_maps, core_ids=list(range(N_CORES)))
    logits = np.stack([r["logits_out"] for r in res.results], axis=0)
    gate = np.stack([r["gate_out"] for r in res.results], axis=0)
    return logits, gate
